# revision 7
# baseline (speedup 1.0000x reference)
"""Causal self-attention (llama-style, RoPE) on 8 Trainium2 NeuronCores.

Problem: B=4, S=2048, D=2048, H=16 heads, HD=128, fp32.
    xq/xk/xv = x @ w{q,k,v}.T ; RoPE(xq, xk) ; causal softmax attention ;
    out = attn @ wo.T ; returns (out, (xk, xv)).

Sharding: 8 cores = batch (4) x head-half (2). Each core handles one batch
and 8 heads: QKV projections for its head slice, flash-style attention, and
a partial output projection (its 1024 hidden channels). Host sums the two
partials per batch and concatenates the xk/xv shards.

Device flow (per core), all matmuls in float32r (full-rate fp32 storage):
  Phase 1a: QT[h] = (wqT-chunk).T @ xT  -> RoPE (rotate-half via a signed
            permutation matmul) -> spilled to DRAM scratch, [hd, s] layout.
  Phase 1bc: K natural = (xT-chunk).T @ wkT -> RoPE (free-dim slicing) ->
            xk output + DRAM scratch; V natural -> xv output + scratch.
            One xT pass for both (s-blocks of 256).
  Phase 2 (per head): scores computed TRANSPOSED, sT[k, q] = KT-chunk.T @ QT,
            so A@V needs no transposes (V natural is the stationary operand
            and exp(sT) streams through the PE). Softmax denominator via a
            ones-vector matmul over the same stream; no max-subtraction
            (scores are O(5), exp is safe in fp32). Causal masking via
            additive -1e9 tiles on the 4 diagonal sub-blocks.
  Phase 3: out[s-chunk, dout] = sum_h attnT[h]-chunk.T @ woT[h] -> partial out,
            with woT streamed per 512-wide dout block.
"""

import numpy as np

import concourse.bacc as bacc
import concourse.mybir as mybir
import concourse.tile as tile
from concourse.bass_utils import run_bass_kernel_spmd
from concourse.masks import make_identity

B, S, D, H = 4, 2048, 2048, 16
HD = 128
NCORES = 8
HPC = H // 2          # heads per core
DHC = HPC * HD        # per-core hidden slice (1024)
KC = D // 128         # contraction chunks (16)
SB = 512              # s-block width for Q projection / attention q-blocks
NSB = S // SB         # 4
SBKV = 256            # s-block width for the merged K+V pass
NSBKV = S // SBKV     # 8
SCN = S // 128        # s-chunks (16)
SCALE = 1.0 / float(np.sqrt(HD))
F32 = mybir.dt.float32
F32R = mybir.dt.float32r

_NC_CACHE = [None]


def build_nc(reps=1):
    """Build + bacc-compile the per-core program (same program on all cores)."""
    nc = bacc.Bacc()

    xT = nc.dram_tensor("xT", [D, S], F32, kind="ExternalInput")
    wqT = nc.dram_tensor("wqT", [D, DHC], F32, kind="ExternalInput")
    wkT = nc.dram_tensor("wkT", [D, DHC], F32, kind="ExternalInput")
    wvT = nc.dram_tensor("wvT", [D, DHC], F32, kind="ExternalInput")
    woT = nc.dram_tensor("woT", [DHC, D], F32, kind="ExternalInput")
    cosT = nc.dram_tensor("cosT", [HD, S], F32, kind="ExternalInput")
    sinT = nc.dram_tensor("sinT", [HD, S], F32, kind="ExternalInput")
    cosN = nc.dram_tensor("cosN", [S, HD], F32, kind="ExternalInput")
    sinS = nc.dram_tensor("sinS", [S, HD], F32, kind="ExternalInput")

    out_p = nc.dram_tensor("out_p", [S, D], F32, kind="ExternalOutput")
    xk_sh = nc.dram_tensor("xk_sh", [S, DHC], F32, kind="ExternalOutput")
    xv_sh = nc.dram_tensor("xv_sh", [S, DHC], F32, kind="ExternalOutput")

    xT3 = xT.rearrange("(kc p) s -> p kc s", p=128).bitcast(F32R)
    wqT3 = wqT.rearrange("(kc p) m -> p kc m", p=128).bitcast(F32R)
    wkT3 = wkT.rearrange("(kc p) m -> p kc m", p=128).bitcast(F32R)
    wvT3 = wvT.rearrange("(kc p) m -> p kc m", p=128).bitcast(F32R)
    woT3 = woT.rearrange("(hc p) n -> p hc n", p=128).bitcast(F32R)
    cosN3 = cosN.rearrange("(sc p) h -> p sc h", p=128)
    sinS3 = sinS.rearrange("(sc p) h -> p sc h", p=128)
    xk3 = xk_sh.rearrange("(sc p) d -> p sc d", p=128)
    xv3 = xv_sh.rearrange("(sc p) d -> p sc d", p=128)
    out3 = out_p.rearrange("(sc p) d -> p sc d", p=128)

    AF = mybir.ActivationFunctionType
    OP = mybir.AluOpType

    with tile.TileContext(nc, pool_alloc_mode="queue") as tc:
        with (
            tc.tile_pool(name="dram", bufs=1, space="DRAM") as dram,
            tc.tile_pool(name="consts", bufs=1) as sing,
        ):
            qts = dram.tile([HPC, HD, S], F32R)        # roped QT per head
            k_scr = dram.tile([S, DHC], F32R)          # roped K, natural
            v_scr = dram.tile([S, DHC], F32R)          # V, natural
            k_scr3 = k_scr.rearrange("(sc p) d -> p sc d", p=128)
            v_scr3 = v_scr.rearrange("(sc p) d -> p sc d", p=128)

            ident_f = sing.tile([128, 128], F32)
            make_identity(nc, ident_f)
            ident = sing.tile([128, 128], F32R)
            nc.scalar.copy(out=ident, in_=ident_f)

            ones_f = sing.tile([128, 1], F32)
            nc.vector.memset(ones_f, 1.0)
            ones = sing.tile([128, 1], F32R)
            nc.scalar.copy(out=ones, in_=ones_f)

            # RT = R^T for rotate-half: rot(q) = R @ q = (RT).T @ q
            rt_f = sing.tile([128, 128], F32)
            nc.gpsimd.memset(rt_f, 0.0)
            nc.gpsimd.affine_select(
                out=rt_f, in_=rt_f, compare_op=OP.not_equal, fill=-1.0,
                base=-64, channel_multiplier=1, pattern=[[-1, 128]])
            nc.gpsimd.affine_select(
                out=rt_f, in_=rt_f, compare_op=OP.not_equal, fill=1.0,
                base=64, channel_multiplier=1, pattern=[[-1, 128]])
            rt = sing.tile([128, 128], F32R)
            nc.scalar.copy(out=rt, in_=rt_f)

            for _rep in range(reps):
                # ---------------- Phase 1a: Q (transposed) ----------------
                with (
                    tc.tile_pool(name="ropeT", bufs=1) as rtp,
                    tc.tile_pool(name="wq", bufs=KC) as wqp,
                    tc.tile_pool(name="x1", bufs=2 * KC) as xp,
                    tc.tile_pool(name="qwork", bufs=2) as qw,
                    tc.tile_pool(name="pq", bufs=2, space="PSUM") as pq,
                    tc.tile_pool(name="pr", bufs=2, space="PSUM") as pr,
                ):
                    wq_c = [None] * KC
                    xt0 = []
                    for kc in range(KC):
                        # interleave first x-block and weight chunks so the
                        # first accumulation isn't starved by bulk weight DMA
                        t = xp.tile([128, SB], F32R, tag="xt", name=f"xt0_{kc}")
                        nc.sync.dma_start(out=t, in_=xT3[:, kc, 0:SB])
                        xt0.append(t)
                        w = wqp.tile([128, DHC], F32R, tag="wq", name=f"wq{kc}")
                        nc.sync.dma_start(out=w, in_=wqT3[:, kc, :])
                        wq_c[kc] = w
                    cosT_sb = rtp.tile([HD, S], F32)
                    nc.sync.dma_start(out=cosT_sb, in_=cosT[:, :])
                    sinT_sb = rtp.tile([HD, S], F32)
                    nc.sync.dma_start(out=sinT_sb, in_=sinT[:, :])
                    for sb_i in range(NSB):
                        ssl = slice(sb_i * SB, (sb_i + 1) * SB)
                        if sb_i == 0:
                            xt = xt0
                        else:
                            xt = []
                            for kc in range(KC):
                                t = xp.tile([128, SB], F32R, tag="xt", name=f"xt{kc}")
                                nc.sync.dma_start(out=t, in_=xT3[:, kc, ssl])
                                xt.append(t)
                        for h in range(HPC):
                            ps_q = pq.tile([128, SB], F32, tag="psq")
                            for kc in range(KC):
                                nc.tensor.matmul(
                                    ps_q, wq_c[kc][:, h * HD:(h + 1) * HD], xt[kc],
                                    start=(kc == 0), stop=(kc == KC - 1))
                            q_r = qw.tile([128, SB], F32R, tag="qr")
                            nc.scalar.copy(out=q_r, in_=ps_q)
                            ps_rot = pr.tile([128, SB], F32, tag="psrot")
                            nc.tensor.matmul(ps_rot, rt, q_r, start=True, stop=True)
                            tcos = qw.tile([128, SB], F32, tag="tcos")
                            nc.vector.tensor_tensor(tcos, ps_q, cosT_sb[:, ssl], OP.mult)
                            tsin = qw.tile([128, SB], F32, tag="tsin")
                            nc.vector.tensor_tensor(tsin, ps_rot, sinT_sb[:, ssl], OP.mult)
                            qro = qw.tile([128, SB], F32R, tag="qro")
                            nc.vector.tensor_tensor(qro, tcos, tsin, OP.add)
                            nc.sync.dma_start(out=qts[h, :, ssl], in_=qro)

                # -------- Phase 1bc: K (RoPE) + V, one xT pass --------
                with (
                    tc.tile_pool(name="x2", bufs=2 * KC) as xp2,
                    tc.tile_pool(name="ropeN", bufs=1) as rnp,
                    tc.tile_pool(name="wk", bufs=KC) as wkp,
                    tc.tile_pool(name="wv", bufs=KC) as wvp,
                    tc.tile_pool(name="kwork", bufs=2) as kw,
                    tc.tile_pool(name="pk", bufs=4, space="PSUM") as pk,
                ):
                    wk_c, wv_c = [], []
                    xt0 = []
                    for kc in range(KC):
                        t = xp2.tile([128, SBKV], F32R, tag="xt2", name=f"x20_{kc}")
                        nc.sync.dma_start(out=t, in_=xT3[:, kc, 0:SBKV])
                        xt0.append(t)
                        w = wkp.tile([128, DHC], F32R, tag="wk", name=f"wk{kc}")
                        nc.sync.dma_start(out=w, in_=wkT3[:, kc, :])
                        wk_c.append(w)
                    for kc in range(KC):
                        w = wvp.tile([128, DHC], F32R, tag="wv", name=f"wv{kc}")
                        nc.sync.dma_start(out=w, in_=wvT3[:, kc, :])
                        wv_c.append(w)
                    cosN_sb = rnp.tile([128, SCN, HD], F32)
                    nc.sync.dma_start(out=cosN_sb, in_=cosN3)
                    sinS_sb = rnp.tile([128, SCN, HD], F32)
                    nc.sync.dma_start(out=sinS_sb, in_=sinS3)
                    for sb_i in range(NSBKV):
                        if sb_i == 0:
                            xt = xt0
                        else:
                            xt = []
                            for kc in range(KC):
                                t = xp2.tile([128, SBKV], F32R, tag="xt2", name=f"xt2_{kc}")
                                nc.sync.dma_start(
                                    out=t,
                                    in_=xT3[:, kc, sb_i * SBKV:(sb_i + 1) * SBKV])
                                xt.append(t)
                        for sc in range(SBKV // 128):
                            scg = (SBKV // 128) * sb_i + sc
                            csl = slice(sc * 128, (sc + 1) * 128)
                            for db in range(2):
                                dsl = slice(db * SB, (db + 1) * SB)
                                # K
                                ps_k = pk.tile([128, 4, HD], F32, tag="psk")
                                for kc in range(KC):
                                    nc.tensor.matmul(
                                        ps_k, xt[kc][:, csl], wk_c[kc][:, dsl],
                                        start=(kc == 0), stop=(kc == KC - 1))
                                cN = cosN_sb[:, scg, None, :].to_broadcast([128, 4, HD])
                                sS0 = sinS_sb[:, scg, None, 0:64].to_broadcast([128, 4, 64])
                                sS1 = sinS_sb[:, scg, None, 64:HD].to_broadcast([128, 4, 64])
                                tc_ = kw.tile([128, 4, HD], F32, tag="ktc")
                                nc.vector.tensor_tensor(tc_, ps_k, cN, OP.mult)
                                ts_ = kw.tile([128, 4, HD], F32, tag="kts")
                                nc.vector.tensor_tensor(
                                    ts_[:, :, 0:64], ps_k[:, :, 64:HD], sS0, OP.mult)
                                nc.vector.tensor_tensor(
                                    ts_[:, :, 64:HD], ps_k[:, :, 0:64], sS1, OP.mult)
                                kro = kw.tile([128, 4, HD], F32R, tag="kro")
                                nc.vector.tensor_tensor(kro, tc_, ts_, OP.add)
                                nc.sync.dma_start(out=xk3[:, scg, dsl], in_=kro.bitcast(F32))
                                nc.sync.dma_start(out=k_scr3[:, scg, dsl], in_=kro)
                                # V
                                ps_v = pk.tile([128, SB], F32, tag="psv")
                                for kc in range(KC):
                                    nc.tensor.matmul(
                                        ps_v, xt[kc][:, csl], wv_c[kc][:, dsl],
                                        start=(kc == 0), stop=(kc == KC - 1))
                                v_r = kw.tile([128, SB], F32R, tag="vr")
                                nc.scalar.copy(out=v_r, in_=ps_v)
                                nc.sync.dma_start(out=xv3[:, scg, dsl], in_=v_r.bitcast(F32))
                                nc.sync.dma_start(out=v_scr3[:, scg, dsl], in_=v_r)

                # ---------------- Phase 2: attention (per head) ----------------
                attn_ctx = tc.tile_pool(name="attn_res", bufs=1)
                attn_pool = attn_ctx.__enter__()
                attnT = attn_pool.tile([128, HPC, S], F32R)  # [hd, h, q]
                with (
                    tc.tile_pool(name="msk", bufs=1) as mskp,
                    tc.tile_pool(name="h2", bufs=2) as h2,
                    tc.tile_pool(name="ktp", bufs=2 * SCN) as ktp,
                    tc.tile_pool(name="expp", bufs=6) as expp,
                    tc.tile_pool(name="small2", bufs=3) as small2,
                    tc.tile_pool(name="ps_sc", bufs=4, space="PSUM") as ps_sc,
                    tc.tile_pool(name="ps_av", bufs=2, space="PSUM") as ps_av,
                    tc.tile_pool(name="ps_den", bufs=1, space="PSUM") as ps_den,
                    tc.tile_pool(name="ps_tr", bufs=1, space="PSUM") as ps_tr,
                ):
                    # additive causal masks for the 4 diagonal 128x512 blocks:
                    # keep iff qf - 128*r - p >= 0 else -1e9
                    dmask = mskp.tile([128, 4, SB], F32)
                    for r in range(4):
                        nc.gpsimd.memset(dmask[:, r, :], 0.0)
                        nc.gpsimd.affine_select(
                            out=dmask[:, r, :], in_=dmask[:, r, :],
                            compare_op=OP.is_ge, fill=-1e9,
                            base=-128 * r, channel_multiplier=-1, pattern=[[1, SB]])
                    for h in range(HPC):
                        hsl = slice(h * HD, (h + 1) * HD)
                        qt_h = h2.tile([128, S], F32R, tag="qt")
                        nc.sync.dma_start(out=qt_h, in_=qts[h, :, :])
                        k_h = h2.tile([128, SCN, HD], F32R, tag="kh")
                        nc.sync.dma_start(out=k_h, in_=k_scr3[:, :, hsl])
                        v_h = h2.tile([128, SCN, HD], F32R, tag="vh")
                        nc.sync.dma_start(out=v_h, in_=v_scr3[:, :, hsl])
                        kt = []
                        for sc in range(SCN):
                            ptr = ps_tr.tile([128, 128], F32R, tag="ptr")
                            nc.tensor.transpose(ptr, k_h[:, sc, :], ident)
                            t = ktp.tile([128, 128], F32R, tag="ktc", name=f"kt{sc}")
                            nc.vector.tensor_copy(out=t, in_=ptr)
                            kt.append(t)
                        for qb in range(4):
                            qsl = slice(qb * SB, (qb + 1) * SB)
                            kbm = 4 * (qb + 1)
                            av = ps_av.tile([128, SB], F32, tag="av")
                            den = ps_den.tile([1, SB], F32, tag="den")
                            for kb in range(kbm):
                                scs = ps_sc.tile([128, SB], F32, tag="scs")
                                nc.tensor.matmul(
                                    scs, kt[kb], qt_h[:, qsl], start=True, stop=True)
                                if kb >= 4 * qb:
                                    nc.vector.tensor_add(
                                        out=scs, in0=scs, in1=dmask[:, kb - 4 * qb, :])
                                ex = expp.tile([128, SB], F32R, tag="ex")
                                nc.scalar.activation(
                                    out=ex, in_=scs, func=AF.Exp, scale=SCALE)
                                nc.tensor.matmul(
                                    av, v_h[:, kb, :], ex,
                                    start=(kb == 0), stop=(kb == kbm - 1))
                                nc.tensor.matmul(
                                    den, ones, ex,
                                    start=(kb == 0), stop=(kb == kbm - 1))
                            recip = small2.tile([1, SB], F32, tag="recip")
                            nc.vector.reciprocal(out=recip, in_=den)
                            rbc = small2.tile([128, SB], F32, tag="rbc")
                            nc.gpsimd.partition_broadcast(rbc, recip)
                            nc.vector.tensor_tensor(attnT[:, h, qsl], av, rbc, OP.mult)

                # ---------------- Phase 3: output projection ----------------
                with (
                    tc.tile_pool(name="wo", bufs=2) as wop,
                    tc.tile_pool(name="ow", bufs=3) as ow,
                    tc.tile_pool(name="ps_o", bufs=2, space="PSUM") as ps_o,
                ):
                    for db in range(4):
                        dsl = slice(db * SB, (db + 1) * SB)
                        wo_db = wop.tile([128, HPC, SB], F32R, tag="wodb")
                        for hc in range(HPC):
                            nc.sync.dma_start(out=wo_db[:, hc, :], in_=woT3[:, hc, dsl])
                        for sc in range(SCN):
                            po = ps_o.tile([128, SB], F32, tag="po")
                            for h in range(HPC):
                                nc.tensor.matmul(
                                    po, attnT[:, h, sc * 128:(sc + 1) * 128],
                                    wo_db[:, h, :],
                                    start=(h == 0), stop=(h == HPC - 1))
                            ob = ow.tile([128, SB], F32, tag="ob")
                            nc.scalar.copy(out=ob, in_=po)
                            nc.sync.dma_start(out=out3[:, sc, dsl], in_=ob)
                attn_ctx.__exit__(None, None, None)

    nc.compile()
    return nc


def make_in_maps(x, wq, wk, wv, wo, freqs_cos, freqs_sin):
    x = np.asarray(x, dtype=np.float32)
    wq = np.asarray(wq, dtype=np.float32)
    wk = np.asarray(wk, dtype=np.float32)
    wv = np.asarray(wv, dtype=np.float32)
    wo = np.asarray(wo, dtype=np.float32)
    fc = np.asarray(freqs_cos, dtype=np.float32)
    fs = np.asarray(freqs_sin, dtype=np.float32)

    cos_cat = np.concatenate([fc, fc], axis=-1)            # [S, HD]
    sin_cat = np.concatenate([fs, fs], axis=-1)            # [S, HD]
    cosT = np.ascontiguousarray(cos_cat.T)                 # [HD, S]
    sinT = np.ascontiguousarray(sin_cat.T)
    cosN = np.ascontiguousarray(cos_cat)                   # [S, HD]
    sinS = np.concatenate([-fs, fs], axis=-1).copy()       # [S, HD], sign-folded

    in_maps = []
    for c in range(NCORES):
        b, half = c // 2, c % 2
        hsl = slice(half * DHC, (half + 1) * DHC)
        in_maps.append({
            "xT": np.ascontiguousarray(x[b].T),
            "wqT": np.ascontiguousarray(wq[hsl, :].T),
            "wkT": np.ascontiguousarray(wk[hsl, :].T),
            "wvT": np.ascontiguousarray(wv[hsl, :].T),
            "woT": np.ascontiguousarray(wo[:, hsl].T),
            "cosT": cosT, "sinT": sinT, "cosN": cosN, "sinS": sinS,
        })
    return in_maps


def assemble(results):
    out = np.empty((B, S, D), dtype=np.float32)
    xk = np.empty((B, S, H, HD), dtype=np.float32)
    xv = np.empty((B, S, H, HD), dtype=np.float32)
    for c in range(NCORES):
        b, half = c // 2, c % 2
        r = results[c]
        if half == 0:
            out[b] = r["out_p"]
        else:
            out[b] += r["out_p"]
        xk[b, :, half * HPC:(half + 1) * HPC, :] = r["xk_sh"].reshape(S, HPC, HD)
        xv[b, :, half * HPC:(half + 1) * HPC, :] = r["xv_sh"].reshape(S, HPC, HD)
    return out, (xk, xv)


def kernel(x, wq, wk, wv, wo, freqs_cos, freqs_sin):
    if _NC_CACHE[0] is None:
        _NC_CACHE[0] = build_nc()
    nc = _NC_CACHE[0]
    in_maps = make_in_maps(x, wq, wk, wv, wo, freqs_cos, freqs_sin)
    res = run_bass_kernel_spmd(nc, in_maps, core_ids=list(range(NCORES)))
    return assemble(res.results)
